# revision 21
# baseline (speedup 1.0000x reference)
"""LogSparse attention kernel for 8 TRN2 NeuronCores.

Problem: B=4, S=2048, H=1024, 16 heads x 64 dim. Logsparse mask: query i
attends key j iff i-j == 0 or i-j == 2^k (so <=12 keys per query, at
power-of-2 offsets).

Sharding: core c -> batch b = c//2, head-group g = c%2 (8 heads each).
Each core computes q/k/v projections for its (batch, head-group) and the
sparse attention, writing out[b, :, g*512:(g+1)*512].

Device algorithm (per core):
  - X is transposed on the HOST and streamed in per-contraction-chunk so
    the first projection matmuls start ~4us in; dummy warmup matmuls are
    interleaved into the DMA-paced ramp to keep the PE HAM clock at 8/8.
  - QT/KT = W @ XT ([dh, s], dh on partitions) with the weight slab
    stationary across 4 consecutive N=512 matmuls (amortizes the PE
    drain-on-weight-swap), V = X @ WvT (s-major, with a ones column for
    row sums). After each 128-row slab of QT/KT, SBUF->SBUF xbar
    transposes produce s-major per-slab copies qs_t/ks_t [s%128, blk,
    128] (whole-tile transposes only: sliced transpose outputs have
    unreliable DMA ordering).
  - Far diagonals (offsets 256/512/1024) only need diag(Q Kshift^T):
    batched DVE products of s-major q/k over all query blocks + one
    segmented tensor_reduce per (slab, offset), exp'd on ACT right after
    each slab, relayed to qb-major pfar2 via one gpsimd copy.
  - Far p*v MACs are per-lane work and MUST stay off gpsimd whenever
    DVE is active (concurrent gpsimd tensor ops slow DVE ops ~3x via
    SBUF port contention); gpsimd only does the tiny pfar relayout. The
    d=2 MAC is ONE batched DVE op hoisted into the DVE-idle window
    right after the projections; d=4/8 run per-qb in the finalize. The
    65-wide (ones-column) vv rows accumulate far rowsums for free AND
    keep the finalize psum read contiguous (260-element runs; slicing
    out the rowsum column makes the psum AP non-contiguous and costs
    5-8us per DVE op).
  - Dense attention is key-block-major: key block kb serves query blocks
    kb and kb+1 (256 score columns; 8 heads as row-tiled matmul pairs in
    two 2-bank psum tiles). All masking is additive -1e9 on the PE via
    identity-stationary matmuls (one kb-invariant [128,2,256] madd tile
    when attention_mask is zero). Two batched exp ACTs per kb.
  - PV: per qb, 16 matmuls (2 strips x 8 heads, N=65 incl rowsum col)
    accumulate into 2 psum banks; the DVE finalize is the d=4/8 MACs,
    one contiguous psum+facc add, a reciprocal, and the normalize
    multiply.
Softmax max-subtraction is skipped: scores*0.125 has std ~0.4 for this
problem family, far from exp overflow.
"""

import numpy as np
import ml_dtypes

import concourse.bass as bass
from concourse import bacc
import concourse.mybir as mybir
from concourse.tile import TileContext
from concourse.bass_utils import run_bass_kernel_spmd

B, S, H = 4, 2048, 1024
NH, HD = 16, 64
G = 2  # head groups per batch
HPC = NH // G  # heads per core = 8
GD = HPC * HD  # 512 group dim
NQB = S // 128  # 16 query blocks
KCH = H // 128  # 8 contraction chunks

BF16 = mybir.dt.bfloat16
F32 = mybir.dt.float32
NPBF16 = ml_dtypes.bfloat16

FAR = (2, 4, 8)  # far diagonal offsets in 128-blocks (== 256/512/1024)


def _allowed(diff):
    return (diff == 0) | ((diff > 0) & ((diff & (diff - 1)) == 0))


def _n_far(qb):
    return sum(1 for d in FAR if qb - d >= 0)


def build_program(has_bias: bool, has_am: bool):
    nc = bacc.Bacc("TRN2", target_bir_lowering=False)

    # host-pretransposed X: xt_d[p, c, s] = X[s, c*128+p]
    xt_d = nc.declare_dram_parameter("xt", [128, KCH, S], BF16, isOutput=False)
    wq_d = nc.declare_dram_parameter("wq", [128, KCH, GD], BF16, isOutput=False)
    wk_d = nc.declare_dram_parameter("wk", [128, KCH, GD], BF16, isOutput=False)
    wv_d = nc.declare_dram_parameter("wv", [128, KCH, GD], BF16, isOutput=False)
    eye_d = nc.declare_dram_parameter("eye", [128, 128], BF16, isOutput=False)
    if has_am:
        # dense ADDITIVE log-masks per key block, replicated x2 so one
        # N=512 matmul (identity stationary) adds them to a whole psum
        # bank: [pj, kb, rep, 256]
        masks_d = nc.declare_dram_parameter(
            "masks", [128, NQB, 2, 256], BF16, isOutput=False
        )
        amt_d = nc.declare_dram_parameter("amt", [128, NQB], F32, isOutput=False)
    else:
        # additive -1e9 mask (kb-invariant), applied to all 4 score
        # banks via identity-stationary matmuls on the PE
        madd_d = nc.declare_dram_parameter(
            "madd", [128, 2, 256], BF16, isOutput=False
        )
    if has_bias:
        bqm_d = nc.declare_dram_parameter("bqm", [1, 4, 128], BF16, isOutput=False)
        bkm_d = nc.declare_dram_parameter("bkm", [1, 4, 128], BF16, isOutput=False)
        bv_d = nc.declare_dram_parameter("bv", [1, GD], BF16, isOutput=False)
        ones_row_d = nc.declare_dram_parameter(
            "ones_row", [1, 512], BF16, isOutput=False
        )
    out_d = nc.declare_dram_parameter("out", [S, GD], F32, isOutput=True)

    with TileContext(nc) as tc:
        with (
            tc.tile_pool(name="const", bufs=1) as const_pool,
            tc.tile_pool(name="big", bufs=1) as big_pool,
            tc.tile_pool(name="far_sb", bufs=3) as far_pool,
        ):
            # ---- resident SBUF tensors ----
            qt = big_pool.tile([128, 4, S], BF16, tag="qt")  # [dh%128, m, s]
            kt = big_pool.tile([128, 4, S], BF16, tag="kt")
            # s-major copies for far diagonals, one tile per dh-slab m so
            # every DMA transpose writes a FULL tile:
            # qs_t[m][p, blk, r] = Q[blk*128+p, m*128+r] (heads 2m, 2m+1)
            qs_t = [
                big_pool.tile([128, NQB, 128], BF16, tag=f"qs{m}", name=f"qs{m}")
                for m in range(4)
            ]
            ks_t = [
                big_pool.tile([128, NQB, 128], BF16, tag=f"ks{m}", name=f"ks{m}")
                for m in range(4)
            ]
            vv = big_pool.tile([128, NQB, HPC, HD + 1], BF16, tag="v")
            # far scores / probs, slab-major [s%128, slab, far_idx, qb, j]
            pfar_s = big_pool.tile([128, 4, 3, NQB, 2], F32, tag="pfar_s")
            pfar = big_pool.tile([128, 4, 3, NQB, 2], BF16, tag="pfar")
            # qb-major copy for the MAC broadcast reads (gpsimd relayout)
            pfar2 = big_pool.tile([128, NQB, 3, HPC], BF16, tag="pfar2")
            # batched d=2 far MACs, computed in the DVE-idle window right
            # after the projections (DVE is the binding engine in the
            # attention phase; this hoists ~9us out of it)
            facc2 = big_pool.tile([128, NQB, HPC, HD + 1], BF16, tag="facc2")
            nc.vector.memset(vv[:, :, :, HD : HD + 1], 1.0)
            eye = const_pool.tile([128, 128], BF16, tag="eye")
            if has_am:
                masks = const_pool.tile([128, NQB, 2, 256], BF16, tag="masks")
                amt = const_pool.tile([128, NQB], F32, tag="amt")
            else:
                madd = const_pool.tile([128, 2, 256], BF16, tag="madd")
            if has_bias:
                bqm = const_pool.tile([1, 4, 128], BF16, tag="bqm")
                bkm = const_pool.tile([1, 4, 128], BF16, tag="bkm")
                bvr = const_pool.tile([1, GD], BF16, tag="bvr")
                ones_row = const_pool.tile([1, 512], BF16, tag="ones_row")

            def _far_scores(m):
                """Far-diagonal scores for dh-slab m (heads 2m, 2m+1):
                per offset d, ONE batched DVE product over all query
                blocks, one segmented reduce over dh, then exp on ACT."""
                for di, d in enumerate(FAR):
                    fprod = far_pool.tile(
                        [128, NQB - d, 2, HD], BF16, tag=f"fprod{d}", name=f"fp{m}_{d}"
                    )
                    nc.vector.tensor_mul(
                        fprod.rearrange("p b h d -> p b (h d)"),
                        qs_t[m][:, d:NQB],
                        ks_t[m][:, 0 : NQB - d],
                    )
                    nc.vector.tensor_reduce(
                        pfar_s[:, m, di, d:NQB, :],
                        fprod[:],
                        axis=mybir.AxisListType.X,
                        op=mybir.AluOpType.add,
                    )
                    if has_am:
                        for qb in range(d, NQB):
                            nc.scalar.activation(
                                pfar[:, m, di, qb, :],
                                pfar_s[:, m, di, qb, :],
                                mybir.ActivationFunctionType.Exp,
                                scale=0.125,
                                bias=amt[:, qb - d : qb - d + 1],
                            )
                    else:
                        nc.scalar.activation(
                            pfar[:, m, di, d:NQB, :],
                            pfar_s[:, m, di, d:NQB, :],
                            mybir.ActivationFunctionType.Exp,
                            scale=0.125,
                        )

            def _far_relayout():
                """one gpsimd software-walk relayout into qb-major pfar2
                for the MAC broadcast reads."""
                nc.gpsimd.tensor_copy(
                    pfar2.rearrange("p q d (m j) -> p q d m j", m=4),
                    pfar.rearrange("p m d q j -> p q d m j"),
                )

            # ---- projections: QT/KT [dh, s] ----
            with (
                tc.tile_pool(name="proj_sb", bufs=1) as proj_pool,
                tc.tile_pool(name="ppsum", bufs=8, space="PSUM") as ppsum,
            ):
                xt = proj_pool.tile([128, KCH, S], BF16, tag="xt")
                wq = proj_pool.tile([128, KCH, GD], BF16, tag="wq")
                wk = proj_pool.tile([128, KCH, GD], BF16, tag="wk")
                wv = proj_pool.tile([128, KCH, GD], BF16, tag="wv")

                # load schedule: wq first (small), then xt streamed
                # per-chunk so the first QK matmuls start ~4us in; wk/wv
                # behind on the other queue; mask tiles last (not needed
                # until the attention phase).
                # the first matmul needs only wq + xt chunk 0; the two
                # HWDGE queues share ~358 GB/s, so everything else queues
                # strictly behind the critical prefix
                nc.sync.dma_start(wq[:], wq_d[:])
                nc.scalar.dma_start(xt[:, 0, :], xt_d[:, 0, :])
                nc.sync.dma_start(xt[:, 1, :], xt_d[:, 1, :])
                nc.scalar.dma_start(xt[:, 2, :], xt_d[:, 2, :])
                nc.sync.dma_start(wk[:], wk_d[:])
                for c in range(3, KCH):
                    (nc.scalar if c % 2 else nc.sync).dma_start(
                        xt[:, c, :], xt_d[:, c, :]
                    )
                nc.scalar.dma_start(wv[:], wv_d[:])
                nc.scalar.dma_start(eye[:], eye_d[:])
                if has_am:
                    nc.scalar.dma_start(masks[:], masks_d[:])
                    nc.scalar.dma_start(amt[:], amt_d[:])
                else:
                    nc.scalar.dma_start(madd[:], madd_d[:])
                if has_bias:
                    nc.scalar.dma_start(bqm[:], bqm_d[:])
                    nc.scalar.dma_start(bkm[:], bkm_d[:])
                    nc.scalar.dma_start(bvr[:], bv_d[:])
                    nc.scalar.dma_start(ones_row[:], ones_row_d[:])

                # PE warmup: dependency-free dummy matmuls that run during
                # the startup DMA wait so HAM reaches 8/8 clock before the
                # projections start; more are interleaved into the
                # DMA-paced ramp below.
                scratch = const_pool.tile([128, 512], BF16, tag="warm")
                nc.vector.memset(scratch[:], 0.0)

                def _warm(n):
                    for _ in range(n):
                        wps = ppsum.tile([128, 512], F32, tag="pp")
                        nc.tensor.matmul(
                            wps[:],
                            scratch[:, 0:128],
                            scratch[:],
                            start=True,
                            stop=True,
                            skip_group_check=True,
                        )

                _warm(14)
                # QK: weight slab stationary shared across the 4 n-chunks
                for m in range(4):  # dh 128-row tiles (2 heads each)
                    for dst, w, bias in ((qt, wq, "q"), (kt, wk, "k")):
                        pss = [
                            ppsum.tile([128, 512], F32, tag="pp", name=f"qk{m}{bias}{n}")
                            for n in range(4)
                        ]
                        for c in range(KCH):
                            for n in range(4):
                                nc.tensor.matmul(
                                    pss[n][:],
                                    w[:, c, m * 128 : (m + 1) * 128],
                                    xt[:, c, n * 512 : (n + 1) * 512],
                                    start=(c == 0),
                                    stop=(c == KCH - 1 and not has_bias),
                                )
                            if m == 0 and bias == "q" and c < 6:
                                # keep PE dense through the DMA-paced ramp
                                _warm(3 if c < 3 else 2)
                        if has_bias:
                            brow = bqm if bias == "q" else bkm
                            for n in range(4):
                                nc.tensor.matmul(
                                    pss[n][:],
                                    brow[:, m, :],
                                    ones_row[:],
                                    start=False,
                                    stop=True,
                                )
                        for n in range(4):
                            nc.scalar.activation(
                                dst[:, m, n * 512 : (n + 1) * 512],
                                pss[n][:],
                                mybir.ActivationFunctionType.Copy,
                            )
                    # stream finished 128-row slabs through the SBUF->SBUF
                    # xbar transpose into s-major tiles; q on the sync
                    # queue, k on the scalar queue so they overlap.
                    nc.sync.dma_start_transpose(qs_t[m][:], qt[:, m, :])
                    nc.scalar.dma_start_transpose(ks_t[m][:], kt[:, m, :])
                    # far-diagonal scores + exp for this slab's two heads
                    _far_scores(m)
                _far_relayout()
                # ---- V [s, dh] ----
                for t in range(NQB):
                    ps = ppsum.tile([128, 512], F32, tag="pp", name=f"v{t}")
                    for c in range(KCH):
                        nc.tensor.matmul(
                            ps[:],
                            xt[:, c, t * 128 : (t + 1) * 128],
                            wv[:, c, :],
                            start=(c == 0),
                            stop=(c == KCH - 1 and not has_bias),
                        )
                    if has_bias:
                        nc.tensor.matmul(
                            ps[:], ones_row[:, :128], bvr[:], start=False, stop=True
                        )
                    nc.scalar.activation(
                        vv[:, t, :, 0:HD], ps[:], mybir.ActivationFunctionType.Copy
                    )
            for lo, hi in ((FAR[0], 10), (10, NQB)):
                nc.vector.tensor_mul(
                    facc2[:, lo:hi],
                    vv[:, lo - FAR[0] : hi - FAR[0], :, :],
                    pfar2[:, lo:hi, 0, :, None].broadcast_to(
                        [128, hi - lo, HPC, HD + 1]
                    ),
                )
            # ---- dense attention (key-block major, heads batched) ----
            # sc tile = 2 psum banks, 4 heads; row-tiled matmul pairs
            # (h even K-rows 0:64, h odd 64:128) land in different banks.
            SLOTMAP = (0, 2, 1, 3)

            def tidx(h):
                return h // 4

            def slot(h):
                return SLOTMAP[h % 4]

            with (
                tc.tile_pool(name="spsum", bufs=2, space="PSUM") as spsum,
                tc.tile_pool(name="opsum", bufs=2, space="PSUM") as opsum,
                tc.tile_pool(name="att_sb", bufs=6) as att_sb,
                tc.tile_pool(name="fin_sb", bufs=8) as fin_sb,
            ):
                strips = {}

                def _pv_finalize(qb):
                    pv = opsum.tile([128, 2, 512], F32, tag="pv")
                    for h in range(HPC):
                        half, idx = h // 4, h % 4
                        nc.tensor.matmul(
                            pv[:, half, idx * 65 : idx * 65 + 65],
                            strips[qb][:, tidx(h), slot(h), 0:128],
                            vv[:, qb, h, :],
                            start=True,
                            stop=(qb == 0),
                            skip_group_check=True,
                        )
                        if qb >= 1:
                            nc.tensor.matmul(
                                pv[:, half, idx * 65 : idx * 65 + 65],
                                strips[qb - 1][:, tidx(h), slot(h), 128:256],
                                vv[:, qb - 1, h, :],
                                start=False,
                                stop=True,
                                skip_group_check=True,
                            )
                    # [si, 2, 4, 65] view of the two psum banks — the
                    # 65-wide runs merge to contiguous 260-element reads
                    pvv = pv[:, :, 0:260].rearrange("p a (i c) -> p a i c", i=4)

                    def v4(ap):  # [128, 8, c] -> [128, 2, 4, c]
                        return ap.rearrange("p (a i) c -> p a i c", a=2)

                    nf = _n_far(qb)
                    posb = fin_sb.tile([128, HPC, HD + 1], F32, tag="posb")
                    if nf:
                        # d=4/8 far p*v MACs on DVE (d=2 is the hoisted
                        # batched facc2; gpsimd tensor ops poison
                        # concurrent DVE ops ~3x, so none run here)
                        acc = facc2[:, qb]
                        if nf >= 2:
                            facc = fin_sb.tile(
                                [128, HPC, HD + 1], BF16, tag="facc"
                            )
                            for di in range(1, nf):
                                mtmp = fin_sb.tile(
                                    [128, HPC, HD + 1], BF16, tag="mtmp"
                                )
                                nc.vector.tensor_mul(
                                    mtmp[:],
                                    vv[:, qb - FAR[di]],
                                    pfar2[:, qb, di, :, None].broadcast_to(
                                        [128, HPC, HD + 1]
                                    ),
                                )
                                nc.vector.tensor_add(facc[:], acc, mtmp[:])
                                acc = facc[:]
                        nc.vector.tensor_add(v4(posb[:]), pvv, v4(acc))
                    else:
                        nc.vector.tensor_copy(v4(posb[:]), pvv)
                    rinv = fin_sb.tile([128, HPC, 1], F32, tag="rinv")
                    nc.vector.reciprocal(rinv[:], posb[:, :, HD : HD + 1])
                    outs_t = fin_sb.tile([128, HPC, HD], F32, tag="outs")
                    nc.vector.tensor_mul(
                        outs_t[:],
                        posb[:, :, 0:HD],
                        rinv[:].broadcast_to([128, HPC, HD]),
                    )
                    nc.sync.dma_start(
                        out_d[qb * 128 : (qb + 1) * 128, :],
                        outs_t.rearrange("p h c -> p (h c)"),
                    )

                for kb in range(NQB):
                    nd = 256 if kb + 1 < NQB else 128
                    scs = [
                        spsum.tile([128, 4, 256], F32, tag="sc", name=f"sc{kb}_{i}")
                        for i in range(2)
                    ]
                    pt = att_sb.tile([128, 2, 4, 256], BF16, tag="pt")
                    for h in range(HPC):
                        mh, p0 = h // 2, (h % 2) * 64
                        nc.tensor.matmul(
                            scs[tidx(h)][:, slot(h), 0:nd],
                            kt[p0 : p0 + 64, mh, kb * 128 : (kb + 1) * 128],
                            qt[p0 : p0 + 64, mh, kb * 128 : kb * 128 + nd],
                            start=(h % 4 < 2),
                            stop=False,
                            skip_group_check=True,
                        )
                    for t in range(2):
                        for bank in range(2):
                            # additive logsparse mask via identity-
                            # stationary matmul
                            rhs = (
                                masks[:, kb, :, 0:nd]
                                if has_am
                                else madd[:, :, 0:nd]
                            )
                            nc.tensor.matmul(
                                scs[t][:, 2 * bank : 2 * bank + 2, 0:nd],
                                eye[:],
                                rhs,
                                start=False,
                                stop=True,
                                skip_group_check=True,
                            )
                    for t in range(2):
                        nc.scalar.activation(
                            pt[:, t, :, 0:nd],
                            scs[t][:, :, 0:nd],
                            mybir.ActivationFunctionType.Exp,
                            scale=0.125,
                        )
                    strips[kb] = pt
                    if kb >= 1:
                        _pv_finalize(kb - 1)
                _pv_finalize(NQB - 1)
    nc.compile()
    return nc


_CACHE = {}


def _get_program(has_bias, has_am):
    key = (has_bias, has_am)
    if key not in _CACHE:
        _CACHE[key] = build_program(has_bias, has_am)
    return _CACHE[key]


def _pat(dlt):
    pi = np.arange(128)[None, :]
    pj = np.arange(128)[:, None]
    return _allowed(dlt * 128 + pi - pj)


def _host_masks(attention_mask_b):
    """Dense ADDITIVE log-mask strips [128, NQB, 2, 256] (f32), added to
    the score psum pre-exp: 0 where allowed else -1e9, plus 8*amask[j]
    (per key j = partition) so exp(0.125*(s+M)) = exp(0.125*s)*exp(am)."""
    pat = {
        dlt: np.where(_pat(dlt), 0.0, -1e9).astype(np.float32) for dlt in (0, 1)
    }
    am8 = 8.0 * attention_mask_b.astype(np.float32)  # [S]
    m = np.full((128, NQB, 256), -1e9, dtype=np.float32)
    for kb in range(NQB):
        amw = am8[kb * 128 : (kb + 1) * 128][:, None]  # [pj, 1]
        m[:, kb, 0:128] = pat[0] + amw
        if kb + 1 < NQB:
            m[:, kb, 128:256] = pat[1] + amw
    return np.repeat(m[:, :, None, :], 2, axis=2)


def _host_madd():
    """Additive -1e9 logsparse mask [128, 2, 256] bf16 (kb-invariant,
    replicated over the 2 slots of a psum bank)."""
    patcat = np.concatenate(
        [np.where(_pat(0), 0.0, -1e9), np.where(_pat(1), 0.0, -1e9)], axis=1
    ).astype(np.float32)
    return np.repeat(patcat[:, None, :], 2, axis=1).astype(NPBF16)


def _build_in_maps(
    hidden_states, attention_mask, Wq, bq, Wk, bk, Wv, bv, has_bias, has_am
):
    # per-batch host-transposed X (shared by the two cores of a batch)
    xts = [
        np.ascontiguousarray(
            hidden_states[b].T.reshape(KCH, 128, S).transpose(1, 0, 2)
        ).astype(NPBF16)
        for b in range(B)
    ]
    eye = np.eye(128, dtype=NPBF16)
    madd = None if has_am else _host_madd()
    in_maps = []
    for c in range(8):
        b, g = c // 2, c % 2
        sl = slice(g * GD, (g + 1) * GD)
        im = {
            "xt": xts[b],
            "wq": np.ascontiguousarray(
                Wq[sl, :].T.reshape(KCH, 128, GD).transpose(1, 0, 2)
            ).astype(NPBF16),
            "wk": np.ascontiguousarray(
                Wk[sl, :].T.reshape(KCH, 128, GD).transpose(1, 0, 2)
            ).astype(NPBF16),
            "wv": np.ascontiguousarray(
                Wv[sl, :].T.reshape(KCH, 128, GD).transpose(1, 0, 2)
            ).astype(NPBF16),
            "eye": eye,
        }
        if has_am:
            im["masks"] = _host_masks(attention_mask[b, 0, 0, :]).astype(NPBF16)
            im["amt"] = np.ascontiguousarray(
                attention_mask[b, 0, 0, :].astype(np.float32).reshape(NQB, 128).T
            )
        else:
            im["madd"] = madd
        if has_bias:
            im["bqm"] = bq[sl].reshape(1, 4, 128).astype(NPBF16)
            im["bkm"] = bk[sl].reshape(1, 4, 128).astype(NPBF16)
            im["bv"] = bv[sl].reshape(1, GD).astype(NPBF16)
            im["ones_row"] = np.ones((1, 512), dtype=NPBF16)
        in_maps.append(im)
    return in_maps


def kernel(hidden_states, attention_mask, Wq, bq, Wk, bk, Wv, bv, _trace=False):
    hidden_states = np.asarray(hidden_states)
    attention_mask = np.asarray(attention_mask)
    Wq, bq = np.asarray(Wq), np.asarray(bq)
    Wk, bk = np.asarray(Wk), np.asarray(bk)
    Wv, bv = np.asarray(Wv), np.asarray(bv)

    has_bias = bool(np.any(bq) or np.any(bk) or np.any(bv))
    has_am = bool(np.any(attention_mask))
    nc = _get_program(has_bias, has_am)
    in_maps = _build_in_maps(
        hidden_states, attention_mask, Wq, bq, Wk, bk, Wv, bv, has_bias, has_am
    )

    kw = {}
    if _trace:
        import os
        import shutil

        shutil.rmtree("/tmp/bass_trace", ignore_errors=True)
        os.makedirs("/tmp/bass_trace", exist_ok=True)
        kw = dict(tmpdir="/tmp/bass_trace")
    res = run_bass_kernel_spmd(nc, in_maps, list(range(8)), trace=_trace, **kw)
    out = np.empty((B, S, H), dtype=np.float32)
    for c in range(8):
        b, g = c // 2, c % 2
        out[b, :, g * GD : (g + 1) * GD] = res.results[c]["out"]
    if _trace:
        return out, res
    return out


# revision 22
# speedup vs baseline: 1.0315x; 1.0315x over previous
"""LogSparse attention kernel for 8 TRN2 NeuronCores.

Problem: B=4, S=2048, H=1024, 16 heads x 64 dim. Logsparse mask: query i
attends key j iff i-j == 0 or i-j == 2^k (so <=12 keys per query, at
power-of-2 offsets).

Sharding: core c -> batch b = c//2, head-group g = c%2 (8 heads each).
Each core computes q/k/v projections for its (batch, head-group) and the
sparse attention, writing out[b, :, g*512:(g+1)*512].

Device algorithm (per core):
  - X is transposed on the HOST and streamed in per-contraction-chunk so
    the first projection matmuls start ~4us in; dummy warmup matmuls are
    interleaved into the DMA-paced ramp to keep the PE HAM clock at 8/8.
  - QT/KT = W @ XT ([dh, s], dh on partitions) with the weight slab
    stationary across 4 consecutive N=512 matmuls (amortizes the PE
    drain-on-weight-swap), V = X @ WvT (s-major, with a ones column for
    row sums). After each 128-row slab of QT/KT, SBUF->SBUF xbar
    transposes produce s-major per-slab copies qs_t/ks_t [s%128, blk,
    128] (whole-tile transposes only: sliced transpose outputs have
    unreliable DMA ordering).
  - Far diagonals (offsets 256/512/1024) only need diag(Q Kshift^T):
    batched DVE products of s-major q/k over all query blocks + one
    segmented tensor_reduce per (slab, offset), exp'd on ACT right after
    each slab, relayed to qb-major pfar2 via one gpsimd copy.
  - Far p*v MACs are per-lane work and MUST stay off gpsimd whenever
    DVE is active (concurrent gpsimd tensor ops slow DVE ops ~3x via
    SBUF port contention); gpsimd only does the tiny pfar relayout. The
    d=2 MAC is ONE batched DVE op hoisted into the DVE-idle window
    right after the projections; d=4/8 run per-qb in the finalize. The
    65-wide (ones-column) vv rows accumulate far rowsums for free AND
    keep the finalize psum read contiguous (260-element runs; slicing
    out the rowsum column makes the psum AP non-contiguous and costs
    5-8us per DVE op).
  - Dense attention is key-block-major: key block kb serves query blocks
    kb and kb+1 (256 score columns; 8 heads as row-tiled matmul pairs in
    two 2-bank psum tiles). All masking is additive -1e9 on the PE via
    identity-stationary matmuls (one kb-invariant [128,2,256] madd tile
    when attention_mask is zero). Two batched exp ACTs per kb.
  - PV: per qb, 16 matmuls (2 strips x 8 heads, N=65 incl rowsum col)
    accumulate into 2 psum banks; the DVE finalize is the d=4/8 MACs,
    one contiguous psum+facc add, a reciprocal, and the normalize
    multiply.
Softmax max-subtraction is skipped: scores*0.125 has std ~0.4 for this
problem family, far from exp overflow.
"""

import numpy as np
import ml_dtypes

import concourse.bass as bass
from concourse import bacc
import concourse.mybir as mybir
from concourse.tile import TileContext
from concourse.bass_utils import run_bass_kernel_spmd

B, S, H = 4, 2048, 1024
NH, HD = 16, 64
G = 2  # head groups per batch
HPC = NH // G  # heads per core = 8
GD = HPC * HD  # 512 group dim
NQB = S // 128  # 16 query blocks
KCH = H // 128  # 8 contraction chunks

BF16 = mybir.dt.bfloat16
F32 = mybir.dt.float32
NPBF16 = ml_dtypes.bfloat16

FAR = (2, 4, 8)  # far diagonal offsets in 128-blocks (== 256/512/1024)


def _allowed(diff):
    return (diff == 0) | ((diff > 0) & ((diff & (diff - 1)) == 0))


def _n_far(qb):
    return sum(1 for d in FAR if qb - d >= 0)


def build_program(has_bias: bool, has_am: bool):
    nc = bacc.Bacc("TRN2", target_bir_lowering=False)

    # host-pretransposed X: xt_d[p, c, s] = X[s, c*128+p]
    xt_d = nc.declare_dram_parameter("xt", [128, KCH, S], BF16, isOutput=False)
    wq_d = nc.declare_dram_parameter("wq", [128, KCH, GD], BF16, isOutput=False)
    wk_d = nc.declare_dram_parameter("wk", [128, KCH, GD], BF16, isOutput=False)
    wv_d = nc.declare_dram_parameter("wv", [128, KCH, GD], BF16, isOutput=False)
    eye_d = nc.declare_dram_parameter("eye", [128, 128], BF16, isOutput=False)
    if has_am:
        # dense ADDITIVE log-masks per key block, replicated x2 so one
        # N=512 matmul (identity stationary) adds them to a whole psum
        # bank: [pj, kb, rep, 256]
        masks_d = nc.declare_dram_parameter(
            "masks", [128, NQB, 2, 256], BF16, isOutput=False
        )
        amt_d = nc.declare_dram_parameter("amt", [128, NQB], F32, isOutput=False)
    else:
        # additive -1e9 mask (kb-invariant), applied to all 4 score
        # banks via identity-stationary matmuls on the PE
        madd_d = nc.declare_dram_parameter(
            "madd", [128, 2, 256], BF16, isOutput=False
        )
    if has_bias:
        bqm_d = nc.declare_dram_parameter("bqm", [1, 4, 128], BF16, isOutput=False)
        bkm_d = nc.declare_dram_parameter("bkm", [1, 4, 128], BF16, isOutput=False)
        bv_d = nc.declare_dram_parameter("bv", [1, GD], BF16, isOutput=False)
        ones_row_d = nc.declare_dram_parameter(
            "ones_row", [1, 512], BF16, isOutput=False
        )
    out_d = nc.declare_dram_parameter("out", [S, GD], F32, isOutput=True)

    with TileContext(nc) as tc:
        with (
            tc.tile_pool(name="const", bufs=1) as const_pool,
            tc.tile_pool(name="big", bufs=1) as big_pool,
            tc.tile_pool(name="far_sb", bufs=3) as far_pool,
        ):
            # ---- resident SBUF tensors ----
            qt = big_pool.tile([128, 4, S], BF16, tag="qt")  # [dh%128, m, s]
            kt = big_pool.tile([128, 4, S], BF16, tag="kt")
            # s-major copies for far diagonals, one tile per dh-slab m so
            # every DMA transpose writes a FULL tile:
            # qs_t[m][p, blk, r] = Q[blk*128+p, m*128+r] (heads 2m, 2m+1)
            qs_t = [
                big_pool.tile([128, NQB, 128], BF16, tag=f"qs{m}", name=f"qs{m}")
                for m in range(4)
            ]
            ks_t = [
                big_pool.tile([128, NQB, 128], BF16, tag=f"ks{m}", name=f"ks{m}")
                for m in range(4)
            ]
            vv = big_pool.tile([128, NQB, HPC, HD + 1], BF16, tag="v")
            # far scores / probs, slab-major [s%128, slab, far_idx, qb, j]
            pfar_s = big_pool.tile([128, 4, 3, NQB, 2], F32, tag="pfar_s")
            pfar = big_pool.tile([128, 4, 3, NQB, 2], BF16, tag="pfar")
            # qb-major copy for the MAC broadcast reads (gpsimd relayout)
            pfar2 = big_pool.tile([128, NQB, 3, HPC], BF16, tag="pfar2")
            # batched d=2 far MACs, computed in the DVE-idle window right
            # after the projections (DVE is the binding engine in the
            # attention phase; this hoists ~9us out of it)
            facc2 = big_pool.tile([128, NQB, HPC, HD + 1], BF16, tag="facc2")
            nc.vector.memset(vv[:, :, :, HD : HD + 1], 1.0)
            eye = const_pool.tile([128, 128], BF16, tag="eye")
            if has_am:
                masks = const_pool.tile([128, NQB, 2, 256], BF16, tag="masks")
                amt = const_pool.tile([128, NQB], F32, tag="amt")
            else:
                madd = const_pool.tile([128, 2, 256], BF16, tag="madd")
            if has_bias:
                bqm = const_pool.tile([1, 4, 128], BF16, tag="bqm")
                bkm = const_pool.tile([1, 4, 128], BF16, tag="bkm")
                bvr = const_pool.tile([1, GD], BF16, tag="bvr")
                ones_row = const_pool.tile([1, 512], BF16, tag="ones_row")

            def _far_scores(m):
                """Far-diagonal scores for dh-slab m (heads 2m, 2m+1):
                per offset d, ONE batched DVE product over all query
                blocks, one segmented reduce over dh, then exp on ACT."""
                for di, d in enumerate(FAR):
                    fprod = far_pool.tile(
                        [128, NQB - d, 2, HD], BF16, tag=f"fprod{d}", name=f"fp{m}_{d}"
                    )
                    nc.vector.tensor_mul(
                        fprod.rearrange("p b h d -> p b (h d)"),
                        qs_t[m][:, d:NQB],
                        ks_t[m][:, 0 : NQB - d],
                    )
                    nc.vector.tensor_reduce(
                        pfar_s[:, m, di, d:NQB, :],
                        fprod[:],
                        axis=mybir.AxisListType.X,
                        op=mybir.AluOpType.add,
                    )
                    if has_am:
                        for qb in range(d, NQB):
                            nc.scalar.activation(
                                pfar[:, m, di, qb, :],
                                pfar_s[:, m, di, qb, :],
                                mybir.ActivationFunctionType.Exp,
                                scale=0.125,
                                bias=amt[:, qb - d : qb - d + 1],
                            )
                    else:
                        nc.scalar.activation(
                            pfar[:, m, di, d:NQB, :],
                            pfar_s[:, m, di, d:NQB, :],
                            mybir.ActivationFunctionType.Exp,
                            scale=0.125,
                        )

            def _far_relayout():
                """one gpsimd software-walk relayout into qb-major pfar2
                for the MAC broadcast reads."""
                nc.gpsimd.tensor_copy(
                    pfar2.rearrange("p q d (m j) -> p q d m j", m=4),
                    pfar.rearrange("p m d q j -> p q d m j"),
                )

            # ---- projections: QT/KT [dh, s] ----
            with (
                tc.tile_pool(name="proj_sb", bufs=1) as proj_pool,
                tc.tile_pool(name="ppsum", bufs=8, space="PSUM") as ppsum,
            ):
                xt = proj_pool.tile([128, KCH, S], BF16, tag="xt")
                wq = proj_pool.tile([128, KCH, GD], BF16, tag="wq")
                wk = proj_pool.tile([128, KCH, GD], BF16, tag="wk")
                wv = proj_pool.tile([128, KCH, GD], BF16, tag="wv")

                # load schedule: wq first (small), then xt streamed
                # per-chunk so the first QK matmuls start ~4us in; wk/wv
                # behind on the other queue; mask tiles last (not needed
                # until the attention phase).
                nc.sync.dma_start(wq[:], wq_d[:])
                for c in range(3):
                    nc.sync.dma_start(xt[:, c, :], xt_d[:, c, :])
                for c in range(3, KCH):
                    nc.scalar.dma_start(xt[:, c, :], xt_d[:, c, :])
                nc.scalar.dma_start(wk[:], wk_d[:])
                nc.scalar.dma_start(wv[:], wv_d[:])
                nc.scalar.dma_start(eye[:], eye_d[:])
                if has_am:
                    nc.scalar.dma_start(masks[:], masks_d[:])
                    nc.scalar.dma_start(amt[:], amt_d[:])
                else:
                    nc.scalar.dma_start(madd[:], madd_d[:])
                if has_bias:
                    nc.scalar.dma_start(bqm[:], bqm_d[:])
                    nc.scalar.dma_start(bkm[:], bkm_d[:])
                    nc.scalar.dma_start(bvr[:], bv_d[:])
                    nc.scalar.dma_start(ones_row[:], ones_row_d[:])

                # PE warmup: dependency-free dummy matmuls that run during
                # the startup DMA wait so HAM reaches 8/8 clock before the
                # projections start; more are interleaved into the
                # DMA-paced ramp below.
                scratch = const_pool.tile([128, 512], BF16, tag="warm")
                nc.vector.memset(scratch[:], 0.0)

                def _warm(n):
                    for _ in range(n):
                        wps = ppsum.tile([128, 512], F32, tag="pp")
                        nc.tensor.matmul(
                            wps[:],
                            scratch[:, 0:128],
                            scratch[:],
                            start=True,
                            stop=True,
                            skip_group_check=True,
                        )

                _warm(14)
                # QK: weight slab stationary shared across the 4 n-chunks
                for m in range(4):  # dh 128-row tiles (2 heads each)
                    for dst, w, bias in ((qt, wq, "q"), (kt, wk, "k")):
                        pss = [
                            ppsum.tile([128, 512], F32, tag="pp", name=f"qk{m}{bias}{n}")
                            for n in range(4)
                        ]
                        for c in range(KCH):
                            for n in range(4):
                                nc.tensor.matmul(
                                    pss[n][:],
                                    w[:, c, m * 128 : (m + 1) * 128],
                                    xt[:, c, n * 512 : (n + 1) * 512],
                                    start=(c == 0),
                                    stop=(c == KCH - 1 and not has_bias),
                                )
                            if m == 0 and bias == "q" and c < 5:
                                # keep PE dense through the DMA-paced ramp
                                _warm(2)
                        if has_bias:
                            brow = bqm if bias == "q" else bkm
                            for n in range(4):
                                nc.tensor.matmul(
                                    pss[n][:],
                                    brow[:, m, :],
                                    ones_row[:],
                                    start=False,
                                    stop=True,
                                )
                        for n in range(4):
                            nc.scalar.activation(
                                dst[:, m, n * 512 : (n + 1) * 512],
                                pss[n][:],
                                mybir.ActivationFunctionType.Copy,
                            )
                    # stream finished 128-row slabs through the SBUF->SBUF
                    # xbar transpose into s-major tiles; q on the sync
                    # queue, k on the scalar queue so they overlap.
                    nc.sync.dma_start_transpose(qs_t[m][:], qt[:, m, :])
                    nc.scalar.dma_start_transpose(ks_t[m][:], kt[:, m, :])
                    # far-diagonal scores + exp for this slab's two heads
                    _far_scores(m)
                _far_relayout()
                # ---- V [s, dh] ----
                for t in range(NQB):
                    ps = ppsum.tile([128, 512], F32, tag="pp", name=f"v{t}")
                    for c in range(KCH):
                        nc.tensor.matmul(
                            ps[:],
                            xt[:, c, t * 128 : (t + 1) * 128],
                            wv[:, c, :],
                            start=(c == 0),
                            stop=(c == KCH - 1 and not has_bias),
                        )
                    if has_bias:
                        nc.tensor.matmul(
                            ps[:], ones_row[:, :128], bvr[:], start=False, stop=True
                        )
                    nc.scalar.activation(
                        vv[:, t, :, 0:HD], ps[:], mybir.ActivationFunctionType.Copy
                    )
            for lo, hi in ((FAR[0], 10), (10, NQB)):
                nc.vector.tensor_mul(
                    facc2[:, lo:hi],
                    vv[:, lo - FAR[0] : hi - FAR[0], :, :],
                    pfar2[:, lo:hi, 0, :, None].broadcast_to(
                        [128, hi - lo, HPC, HD + 1]
                    ),
                )
            # ---- dense attention (key-block major, heads batched) ----
            # sc tile = 2 psum banks, 4 heads; row-tiled matmul pairs
            # (h even K-rows 0:64, h odd 64:128) land in different banks.
            SLOTMAP = (0, 2, 1, 3)

            def tidx(h):
                return h // 4

            def slot(h):
                return SLOTMAP[h % 4]

            with (
                tc.tile_pool(name="spsum", bufs=2, space="PSUM") as spsum,
                tc.tile_pool(name="opsum", bufs=2, space="PSUM") as opsum,
                tc.tile_pool(name="att_sb", bufs=6) as att_sb,
                tc.tile_pool(name="fin_sb", bufs=8) as fin_sb,
            ):
                strips = {}

                def _pv_finalize(qb):
                    pv = opsum.tile([128, 2, 512], F32, tag="pv")
                    for h in range(HPC):
                        half, idx = h // 4, h % 4
                        nc.tensor.matmul(
                            pv[:, half, idx * 65 : idx * 65 + 65],
                            strips[qb][:, tidx(h), slot(h), 0:128],
                            vv[:, qb, h, :],
                            start=True,
                            stop=(qb == 0),
                            skip_group_check=True,
                        )
                        if qb >= 1:
                            nc.tensor.matmul(
                                pv[:, half, idx * 65 : idx * 65 + 65],
                                strips[qb - 1][:, tidx(h), slot(h), 128:256],
                                vv[:, qb - 1, h, :],
                                start=False,
                                stop=True,
                                skip_group_check=True,
                            )
                    # [si, 2, 4, 65] view of the two psum banks — the
                    # 65-wide runs merge to contiguous 260-element reads
                    pvv = pv[:, :, 0:260].rearrange("p a (i c) -> p a i c", i=4)

                    def v4(ap):  # [128, 8, c] -> [128, 2, 4, c]
                        return ap.rearrange("p (a i) c -> p a i c", a=2)

                    nf = _n_far(qb)
                    posb = fin_sb.tile([128, HPC, HD + 1], F32, tag="posb")
                    if nf:
                        # d=4/8 far p*v MACs on DVE (d=2 is the hoisted
                        # batched facc2; gpsimd tensor ops poison
                        # concurrent DVE ops ~3x, so none run here)
                        acc = facc2[:, qb]
                        if nf >= 2:
                            facc = fin_sb.tile(
                                [128, HPC, HD + 1], BF16, tag="facc"
                            )
                            for di in range(1, nf):
                                mtmp = fin_sb.tile(
                                    [128, HPC, HD + 1], BF16, tag="mtmp"
                                )
                                nc.vector.tensor_mul(
                                    mtmp[:],
                                    vv[:, qb - FAR[di]],
                                    pfar2[:, qb, di, :, None].broadcast_to(
                                        [128, HPC, HD + 1]
                                    ),
                                )
                                nc.vector.tensor_add(facc[:], acc, mtmp[:])
                                acc = facc[:]
                        nc.vector.tensor_add(v4(posb[:]), pvv, v4(acc))
                    else:
                        nc.vector.tensor_copy(v4(posb[:]), pvv)
                    rinv = fin_sb.tile([128, HPC, 1], F32, tag="rinv")
                    nc.vector.reciprocal(rinv[:], posb[:, :, HD : HD + 1])
                    outs_t = fin_sb.tile([128, HPC, HD], F32, tag="outs")
                    nc.vector.tensor_mul(
                        outs_t[:],
                        posb[:, :, 0:HD],
                        rinv[:].broadcast_to([128, HPC, HD]),
                    )
                    nc.sync.dma_start(
                        out_d[qb * 128 : (qb + 1) * 128, :],
                        outs_t.rearrange("p h c -> p (h c)"),
                    )

                for kb in range(NQB):
                    nd = 256 if kb + 1 < NQB else 128
                    scs = [
                        spsum.tile([128, 4, 256], F32, tag="sc", name=f"sc{kb}_{i}")
                        for i in range(2)
                    ]
                    pt = att_sb.tile([128, 2, 4, 256], BF16, tag="pt")
                    for h in range(HPC):
                        mh, p0 = h // 2, (h % 2) * 64
                        nc.tensor.matmul(
                            scs[tidx(h)][:, slot(h), 0:nd],
                            kt[p0 : p0 + 64, mh, kb * 128 : (kb + 1) * 128],
                            qt[p0 : p0 + 64, mh, kb * 128 : kb * 128 + nd],
                            start=(h % 4 < 2),
                            stop=False,
                            skip_group_check=True,
                        )
                    for t in range(2):
                        for bank in range(2):
                            # additive logsparse mask via identity-
                            # stationary matmul
                            rhs = (
                                masks[:, kb, :, 0:nd]
                                if has_am
                                else madd[:, :, 0:nd]
                            )
                            nc.tensor.matmul(
                                scs[t][:, 2 * bank : 2 * bank + 2, 0:nd],
                                eye[:],
                                rhs,
                                start=False,
                                stop=True,
                                skip_group_check=True,
                            )
                    for t in range(2):
                        nc.scalar.activation(
                            pt[:, t, :, 0:nd],
                            scs[t][:, :, 0:nd],
                            mybir.ActivationFunctionType.Exp,
                            scale=0.125,
                        )
                    strips[kb] = pt
                    if kb >= 1:
                        _pv_finalize(kb - 1)
                _pv_finalize(NQB - 1)
    nc.compile()
    return nc


_CACHE = {}


def _get_program(has_bias, has_am):
    key = (has_bias, has_am)
    if key not in _CACHE:
        _CACHE[key] = build_program(has_bias, has_am)
    return _CACHE[key]


def _pat(dlt):
    pi = np.arange(128)[None, :]
    pj = np.arange(128)[:, None]
    return _allowed(dlt * 128 + pi - pj)


def _host_masks(attention_mask_b):
    """Dense ADDITIVE log-mask strips [128, NQB, 2, 256] (f32), added to
    the score psum pre-exp: 0 where allowed else -1e9, plus 8*amask[j]
    (per key j = partition) so exp(0.125*(s+M)) = exp(0.125*s)*exp(am)."""
    pat = {
        dlt: np.where(_pat(dlt), 0.0, -1e9).astype(np.float32) for dlt in (0, 1)
    }
    am8 = 8.0 * attention_mask_b.astype(np.float32)  # [S]
    m = np.full((128, NQB, 256), -1e9, dtype=np.float32)
    for kb in range(NQB):
        amw = am8[kb * 128 : (kb + 1) * 128][:, None]  # [pj, 1]
        m[:, kb, 0:128] = pat[0] + amw
        if kb + 1 < NQB:
            m[:, kb, 128:256] = pat[1] + amw
    return np.repeat(m[:, :, None, :], 2, axis=2)


def _host_madd():
    """Additive -1e9 logsparse mask [128, 2, 256] bf16 (kb-invariant,
    replicated over the 2 slots of a psum bank)."""
    patcat = np.concatenate(
        [np.where(_pat(0), 0.0, -1e9), np.where(_pat(1), 0.0, -1e9)], axis=1
    ).astype(np.float32)
    return np.repeat(patcat[:, None, :], 2, axis=1).astype(NPBF16)


def _build_in_maps(
    hidden_states, attention_mask, Wq, bq, Wk, bk, Wv, bv, has_bias, has_am
):
    # per-batch host-transposed X (shared by the two cores of a batch)
    xts = [
        np.ascontiguousarray(
            hidden_states[b].T.reshape(KCH, 128, S).transpose(1, 0, 2)
        ).astype(NPBF16)
        for b in range(B)
    ]
    eye = np.eye(128, dtype=NPBF16)
    madd = None if has_am else _host_madd()
    in_maps = []
    for c in range(8):
        b, g = c // 2, c % 2
        sl = slice(g * GD, (g + 1) * GD)
        im = {
            "xt": xts[b],
            "wq": np.ascontiguousarray(
                Wq[sl, :].T.reshape(KCH, 128, GD).transpose(1, 0, 2)
            ).astype(NPBF16),
            "wk": np.ascontiguousarray(
                Wk[sl, :].T.reshape(KCH, 128, GD).transpose(1, 0, 2)
            ).astype(NPBF16),
            "wv": np.ascontiguousarray(
                Wv[sl, :].T.reshape(KCH, 128, GD).transpose(1, 0, 2)
            ).astype(NPBF16),
            "eye": eye,
        }
        if has_am:
            im["masks"] = _host_masks(attention_mask[b, 0, 0, :]).astype(NPBF16)
            im["amt"] = np.ascontiguousarray(
                attention_mask[b, 0, 0, :].astype(np.float32).reshape(NQB, 128).T
            )
        else:
            im["madd"] = madd
        if has_bias:
            im["bqm"] = bq[sl].reshape(1, 4, 128).astype(NPBF16)
            im["bkm"] = bk[sl].reshape(1, 4, 128).astype(NPBF16)
            im["bv"] = bv[sl].reshape(1, GD).astype(NPBF16)
            im["ones_row"] = np.ones((1, 512), dtype=NPBF16)
        in_maps.append(im)
    return in_maps


def kernel(hidden_states, attention_mask, Wq, bq, Wk, bk, Wv, bv, _trace=False):
    hidden_states = np.asarray(hidden_states)
    attention_mask = np.asarray(attention_mask)
    Wq, bq = np.asarray(Wq), np.asarray(bq)
    Wk, bk = np.asarray(Wk), np.asarray(bk)
    Wv, bv = np.asarray(Wv), np.asarray(bv)

    has_bias = bool(np.any(bq) or np.any(bk) or np.any(bv))
    has_am = bool(np.any(attention_mask))
    nc = _get_program(has_bias, has_am)
    in_maps = _build_in_maps(
        hidden_states, attention_mask, Wq, bq, Wk, bk, Wv, bv, has_bias, has_am
    )

    kw = {}
    if _trace:
        import os
        import shutil

        shutil.rmtree("/tmp/bass_trace", ignore_errors=True)
        os.makedirs("/tmp/bass_trace", exist_ok=True)
        kw = dict(tmpdir="/tmp/bass_trace")
    res = run_bass_kernel_spmd(nc, in_maps, list(range(8)), trace=_trace, **kw)
    out = np.empty((B, S, H), dtype=np.float32)
    for c in range(8):
        b, g = c // 2, c % 2
        out[b, :, g * GD : (g + 1) * GD] = res.results[c]["out"]
    if _trace:
        return out, res
    return out
